# revision 7
# baseline (speedup 1.0000x reference)
"""SSM kernel for Trainium2 (8 NeuronCores, batch-data-parallel).

The reference network is a linear SSM:
    x_t  = ctrl_t^T P            (per frame t)
    xb_t = x_t B ;  xd_t = x_t D
    s_{t+1} = s_t A + xb_t ;  c_t = (s_t A) C
    out_t = c_t + xd_t  ->  hann window  ->  overlap-add (hop 512)

Because ||A||_2 ~ 0.37, contributions of xb_{t-k} to out_t decay ~5x per
step, so the whole network collapses (to ~1e-6 relative) into a short
causal convolution over frames whose taps absorb the projection, the
scan, the readout, the direct path, the hann window AND the overlap-add:

    signal[b, 512*t : 512*(t+1)] = sum_j ctrl[b, :, t-j]^T @ U_j

with U_j (256 x 512) precomputed on the host (weights-only math) from
V_0 = P@D, V_k = P@B@A^k@C:
    U_j = V_j[:, :512]*w1 + V_{j-1}[:, 512:]*w2.

On-device work is then a single dense matmul sweep: for each 128-frame
block the PSUM tile (128 frames x 512 samples) accumulates
TAPS x 2 matmuls with K=128 (cpd halves), N=512, using float32r
(full-rate FP22) and sliding lhsT windows into the SBUF-resident
control tensor. Batch is sharded 16-per-core across the 8 cores.
"""

import numpy as np

NCORES = 8
BATCH, CPD, FRAMES, HOP = 128, 256, 512, 512
BPC = BATCH // NCORES          # batch per core
TAPS = 6                       # conv depth (truncation err ~4e-5)
PAD = 16                       # zero frames prepended (>= TAPS-1)
FP = FRAMES + PAD
WBLK = FRAMES // 128           # 128-frame output blocks per batch elem

_CACHE = {}


def _build_nc():
    from concourse import bacc, tile, mybir

    f32 = mybir.dt.float32
    f32r = mybir.dt.float32r

    nc = bacc.Bacc("TRN2", target_bir_lowering=False, debug=False,
                   num_devices=NCORES)
    ctrl_d = nc.dram_tensor("ctrl", [2, 128, BPC, FP], f32r,
                            kind="ExternalInput").ap()
    taps_d = nc.dram_tensor("taps", [128, TAPS, 2, HOP], f32r,
                            kind="ExternalInput").ap()
    out_d = nc.dram_tensor("out", [BPC, FRAMES, HOP], f32,
                           kind="ExternalOutput").ap()

    with tile.TileContext(nc) as tc:
        with (
            tc.tile_pool(name="const", bufs=1) as const_pool,
            tc.tile_pool(name="psum", bufs=8, space="PSUM") as psum_pool,
            tc.tile_pool(name="stage", bufs=6) as stage_pool,
        ):
            # issue order: the first output block (b=0, w=0) only needs tap
            # tiles and a 144-frame slice of batch-0 control — land those
            # first so matmuls start ASAP, then stream the bulk.
            u_sb = const_pool.tile([128, TAPS, 2, HOP], f32r)
            ctrl_sb = const_pool.tile([128, 2, BPC, FP], f32r)
            head = 128 + PAD
            for h in range(2):
                nc.sync.dma_start(out=ctrl_sb[:, h, 0, :head],
                                  in_=ctrl_d[h, :, 0, :head])
            for j in range(TAPS):
                for h in range(2):
                    nc.sync.dma_start(out=u_sb[:, j, h, :],
                                      in_=taps_d[:, j, h, :])
            for h in range(2):
                nc.sync.dma_start(out=ctrl_sb[:, h, 0, head:],
                                  in_=ctrl_d[h, :, 0, head:])
            for b in range(1, BPC):
                for h in range(2):
                    nc.sync.dma_start(out=ctrl_sb[:, h, b, :],
                                      in_=ctrl_d[h, :, b, :])

            idx = 0
            for b in range(BPC):
                for w in range(WBLK):
                    ps = psum_pool.tile([128, HOP], f32)
                    base = 128 * w + PAD
                    n_mm = TAPS * 2
                    k = 0
                    for j in range(TAPS):
                        for h in range(2):
                            lhsT = ctrl_sb[:, h, b,
                                           base - j:base - j + 128]
                            rhs = u_sb[:, j, h, :]
                            nc.tensor.matmul(
                                ps[:],
                                lhsT,
                                rhs,
                                start=(k == 0),
                                stop=(k == n_mm - 1),
                            )
                            k += 1
                    st = stage_pool.tile([128, HOP], f32)
                    if idx % 2 == 0:
                        nc.vector.tensor_copy(st[:], ps[:])
                    else:
                        nc.scalar.copy(st[:], ps[:])
                    nc.sync.dma_start(out=out_d[b, 128 * w:128 * (w + 1), :],
                                      in_=st[:])
                    idx += 1

    nc.compile()
    return nc


def _build_taps(proj, state_matrix, input_matrix, output_matrix,
                direct_matrix):
    P = np.asarray(proj, np.float64)
    A = np.asarray(state_matrix, np.float64)
    B = np.asarray(input_matrix, np.float64)
    C = np.asarray(output_matrix, np.float64)
    D = np.asarray(direct_matrix, np.float64)

    n = 2 * HOP
    w = 0.5 * (1.0 - np.cos(2.0 * np.pi * np.arange(n) / n))
    w1, w2 = w[:HOP], w[HOP:]

    V = [P @ D]
    PBAk = P @ B
    for _ in range(1, TAPS):
        PBAk = PBAk @ A
        V.append(PBAk @ C)

    U = np.zeros((TAPS, CPD, HOP))
    for j in range(TAPS):
        U[j] += V[j][:, :HOP] * w1
        if j >= 1:
            U[j] += V[j - 1][:, HOP:] * w2
    # k-tile layout: [cpd_within_half(128 part), tap, half, sample]
    return np.ascontiguousarray(
        U.reshape(TAPS, 2, 128, HOP).transpose(2, 0, 1, 3)
    ).astype(np.float32)


def kernel(control, proj, state_matrix, input_matrix, output_matrix,
           direct_matrix):
    from concourse.bass_utils import run_bass_kernel_spmd

    taps = _build_taps(proj, state_matrix, input_matrix, output_matrix,
                       direct_matrix)

    # control (B, CPD, F) -> per core [half, cpd128, b, frame+pad]
    c = np.asarray(control, np.float32)
    c = c.reshape(NCORES, BPC, 2, 128, FRAMES).transpose(0, 2, 3, 1, 4)
    ctrl_prep = np.zeros((NCORES, 2, 128, BPC, FP), np.float32)
    ctrl_prep[..., PAD:] = c

    if "nc" not in _CACHE:
        _CACHE["nc"] = _build_nc()
    nc = _CACHE["nc"]

    in_maps = [{"ctrl": ctrl_prep[i], "taps": taps} for i in range(NCORES)]
    res = run_bass_kernel_spmd(nc, in_maps, list(range(NCORES)))
    outs = [res.results[i]["out"] for i in range(NCORES)]
    sig = np.concatenate(outs, axis=0).reshape(BATCH, 1, FRAMES * HOP)
    return sig.astype(np.float32)


# revision 11
# speedup vs baseline: 1.0071x; 1.0071x over previous
"""SSM kernel for Trainium2 (8 NeuronCores, batch-data-parallel).

The reference network is a linear SSM:
    x_t  = ctrl_t^T P            (per frame t)
    xb_t = x_t B ;  xd_t = x_t D
    s_{t+1} = s_t A + xb_t ;  c_t = (s_t A) C
    out_t = c_t + xd_t  ->  hann window  ->  overlap-add (hop 512)

Because ||A||_2 ~ 0.37, contributions of xb_{t-k} to out_t decay ~5x per
step, so the whole network collapses (to ~1e-6 relative) into a short
causal convolution over frames whose taps absorb the projection, the
scan, the readout, the direct path, the hann window AND the overlap-add:

    signal[b, 512*t : 512*(t+1)] = sum_j ctrl[b, :, t-j]^T @ U_j

with U_j (256 x 512) precomputed on the host (weights-only math) from
V_0 = P@D, V_k = P@B@A^k@C:
    U_j = V_j[:, :512]*w1 + V_{j-1}[:, 512:]*w2.

On-device work is then a single dense matmul sweep: for each 128-frame
block the PSUM tile (128 frames x 512 samples) accumulates
TAPS x 2 matmuls with K=128 (cpd halves), N=512, using float32r
(full-rate FP22) and sliding lhsT windows into the SBUF-resident
control tensor. Batch is sharded 16-per-core across the 8 cores.
"""

import numpy as np

NCORES = 8
BATCH, CPD, FRAMES, HOP = 128, 256, 512, 512
BPC = BATCH // NCORES          # batch per core
TAPS = 6                       # conv depth (truncation err ~4e-5)
PAD = 16                       # zero frames prepended (>= TAPS-1)
FP = FRAMES + PAD
WBLK = FRAMES // 128           # 128-frame output blocks per batch elem

_CACHE = {}


def _build_nc():
    from concourse import bacc, tile, mybir

    f32 = mybir.dt.float32
    f32r = mybir.dt.float32r

    nc = bacc.Bacc("TRN2", target_bir_lowering=False, debug=False,
                   num_devices=NCORES)
    ctrl_d = nc.dram_tensor("ctrl", [2, 128, BPC, FP], f32r,
                            kind="ExternalInput").ap()
    taps_d = nc.dram_tensor("taps", [128, TAPS, 2, HOP], f32r,
                            kind="ExternalInput").ap()
    out_d = nc.dram_tensor("out", [BPC, FRAMES, HOP], f32,
                           kind="ExternalOutput").ap()

    with tile.TileContext(nc) as tc:
        with (
            tc.tile_pool(name="const", bufs=1) as const_pool,
            tc.tile_pool(name="psum", bufs=1, space="PSUM") as psum_pool,
            tc.tile_pool(name="stage", bufs=6) as stage_pool,
        ):
            # DMA issue order: tap j=0 first, then the two control batches
            # the first wave needs, then remaining taps in consumption
            # order, then the bulk of control.
            u_sb = const_pool.tile([128, TAPS, 2, HOP], f32r)
            ctrl_sb = const_pool.tile([128, 2, BPC, FP], f32r)
            for h in range(2):
                nc.sync.dma_start(out=u_sb[:, 0, h, :], in_=taps_d[:, 0, h, :])
            for b in range(2):
                for h in range(2):
                    nc.sync.dma_start(out=ctrl_sb[:, h, b, :],
                                      in_=ctrl_d[h, :, b, :])
            for j in range(1, TAPS):
                for h in range(2):
                    nc.sync.dma_start(out=u_sb[:, j, h, :],
                                      in_=taps_d[:, j, h, :])
            for b in range(2, BPC):
                for h in range(2):
                    nc.sync.dma_start(out=ctrl_sb[:, h, b, :],
                                      in_=ctrl_d[h, :, b, :])

            # Compute in waves of 8 concurrent PSUM groups, emitting
            # matmuls j-major across the wave: each tap tile feeds 16
            # back-to-back matmuls as soon as it lands, so the upload
            # ramp stays PE-bound instead of stalling group-by-group.
            groups = [(b, w) for b in range(BPC) for w in range(WBLK)]
            WAVE = 8
            n_mm = TAPS * 2
            for wave_start in range(0, len(groups), WAVE):
                wave = groups[wave_start:wave_start + WAVE]
                pss = [psum_pool.tile([128, HOP], f32, name=f"ps{g}",
                                      tag=f"ps{g}")
                       for g in range(len(wave))]
                k = 0
                for j in range(TAPS):
                    for h in range(2):
                        for g, (b, w) in enumerate(wave):
                            base = 128 * w + PAD
                            lhsT = ctrl_sb[:, h, b,
                                           base - j:base - j + 128]
                            nc.tensor.matmul(
                                pss[g][:],
                                lhsT,
                                u_sb[:, j, h, :],
                                start=(k == 0),
                                stop=(k == n_mm - 1),
                            )
                        k += 1
                for g, (b, w) in enumerate(wave):
                    st = stage_pool.tile([128, HOP], f32)
                    if g % 2 == 0:
                        nc.vector.tensor_copy(st[:], pss[g][:])
                    else:
                        nc.scalar.copy(st[:], pss[g][:])
                    nc.sync.dma_start(out=out_d[b, 128 * w:128 * (w + 1), :],
                                      in_=st[:])

    nc.compile()
    return nc


def _build_taps(proj, state_matrix, input_matrix, output_matrix,
                direct_matrix):
    P = np.asarray(proj, np.float64)
    A = np.asarray(state_matrix, np.float64)
    B = np.asarray(input_matrix, np.float64)
    C = np.asarray(output_matrix, np.float64)
    D = np.asarray(direct_matrix, np.float64)

    n = 2 * HOP
    w = 0.5 * (1.0 - np.cos(2.0 * np.pi * np.arange(n) / n))
    w1, w2 = w[:HOP], w[HOP:]

    V = [P @ D]
    PBAk = P @ B
    for _ in range(1, TAPS):
        PBAk = PBAk @ A
        V.append(PBAk @ C)

    U = np.zeros((TAPS, CPD, HOP))
    for j in range(TAPS):
        U[j] += V[j][:, :HOP] * w1
        if j >= 1:
            U[j] += V[j - 1][:, HOP:] * w2
    # k-tile layout: [cpd_within_half(128 part), tap, half, sample]
    return np.ascontiguousarray(
        U.reshape(TAPS, 2, 128, HOP).transpose(2, 0, 1, 3)
    ).astype(np.float32)


def kernel(control, proj, state_matrix, input_matrix, output_matrix,
           direct_matrix):
    from concourse.bass_utils import run_bass_kernel_spmd

    taps = _build_taps(proj, state_matrix, input_matrix, output_matrix,
                       direct_matrix)

    # control (B, CPD, F) -> per core [half, cpd128, b, frame+pad]
    c = np.asarray(control, np.float32)
    c = c.reshape(NCORES, BPC, 2, 128, FRAMES).transpose(0, 2, 3, 1, 4)
    ctrl_prep = np.zeros((NCORES, 2, 128, BPC, FP), np.float32)
    ctrl_prep[..., PAD:] = c

    if "nc" not in _CACHE:
        _CACHE["nc"] = _build_nc()
    nc = _CACHE["nc"]

    in_maps = [{"ctrl": ctrl_prep[i], "taps": taps} for i in range(NCORES)]
    res = run_bass_kernel_spmd(nc, in_maps, list(range(NCORES)))
    outs = [res.results[i]["out"] for i in range(NCORES)]
    sig = np.concatenate(outs, axis=0).reshape(BATCH, 1, FRAMES * HOP)
    return sig.astype(np.float32)


# revision 13
# speedup vs baseline: 1.0220x; 1.0147x over previous
"""SSM kernel for Trainium2 (8 NeuronCores, batch-data-parallel).

The reference network is a linear SSM:
    x_t  = ctrl_t^T P            (per frame t)
    xb_t = x_t B ;  xd_t = x_t D
    s_{t+1} = s_t A + xb_t ;  c_t = (s_t A) C
    out_t = c_t + xd_t  ->  hann window  ->  overlap-add (hop 512)

Because ||A||_2 ~ 0.37, contributions of xb_{t-k} to out_t decay ~5x per
step, so the whole network collapses (to ~1e-6 relative) into a short
causal convolution over frames whose taps absorb the projection, the
scan, the readout, the direct path, the hann window AND the overlap-add:

    signal[b, 512*t : 512*(t+1)] = sum_j ctrl[b, :, t-j]^T @ U_j

with U_j (256 x 512) precomputed on the host (weights-only math) from
V_0 = P@D, V_k = P@B@A^k@C:
    U_j = V_j[:, :512]*w1 + V_{j-1}[:, 512:]*w2.

On-device work is then a single dense matmul sweep: for each 128-frame
block the PSUM tile (128 frames x 512 samples) accumulates
TAPS x 2 matmuls with K=128 (cpd halves), N=512, using float32r
(full-rate FP22) and sliding lhsT windows into the SBUF-resident
control tensor. Batch is sharded 16-per-core across the 8 cores.
"""

import numpy as np

NCORES = 8
BATCH, CPD, FRAMES, HOP = 128, 256, 512, 512
BPC = BATCH // NCORES          # batch per core
TAPS = 6                       # conv depth (truncation err ~4e-5)
PAD = 16                       # zero frames prepended (>= TAPS-1)
FP = FRAMES + PAD
WBLK = FRAMES // 128           # 128-frame output blocks per batch elem

_CACHE = {}


def _build_nc():
    from concourse import bacc, tile, mybir

    f32 = mybir.dt.float32
    f32r = mybir.dt.float32r

    nc = bacc.Bacc("TRN2", target_bir_lowering=False, debug=False,
                   num_devices=NCORES)
    ctrl_d = nc.dram_tensor("ctrl", [2, 128, BPC, FP], f32r,
                            kind="ExternalInput").ap()
    taps_d = nc.dram_tensor("taps", [128, TAPS, 2, HOP], f32r,
                            kind="ExternalInput").ap()
    out_d = nc.dram_tensor("out", [BPC, FRAMES, HOP], f32,
                           kind="ExternalOutput").ap()

    with tile.TileContext(nc) as tc:
        with (
            tc.tile_pool(name="const", bufs=1) as const_pool,
            tc.tile_pool(name="psum", bufs=1, space="PSUM") as psum_pool,
            tc.tile_pool(name="stage", bufs=6) as stage_pool,
        ):
            # DMA issue order: tap j=0 first, then the two control batches
            # the first wave needs, then remaining taps in consumption
            # order, then the bulk of control.
            u_sb = const_pool.tile([128, TAPS, 2, HOP], f32r)
            ctrl_sb = const_pool.tile([128, 2, BPC, FP], f32r)
            def dma_u(j):
                for h in range(2):
                    nc.sync.dma_start(out=u_sb[:, j, h, :],
                                      in_=taps_d[:, j, h, :])

            def dma_ctrl(b):
                for h in range(2):
                    nc.sync.dma_start(out=ctrl_sb[:, h, b, :],
                                      in_=ctrl_d[h, :, b, :])

            dma_u(0)
            dma_ctrl(0)
            dma_u(1)
            dma_ctrl(1)
            for j in range(2, TAPS):
                dma_u(j)
            for b in range(2, BPC):
                dma_ctrl(b)

            # Compute in waves of 8 concurrent PSUM groups. While inputs
            # are still streaming in (first 2 waves), emit matmuls
            # j-major across the wave so each arriving tap tile feeds 16
            # back-to-back matmuls and the ramp stays PE-bound. Once all
            # data is resident, emit group-major so PSUM drains spread
            # out evenly (shorter kernel tail).
            groups = [(b, w) for b in range(BPC) for w in range(WBLK)]
            WAVE = 8
            n_mm = TAPS * 2

            def emit_mm(ps, b, w, j, h, k):
                base = 128 * w + PAD
                nc.tensor.matmul(
                    ps[:],
                    ctrl_sb[:, h, b, base - j:base - j + 128],
                    u_sb[:, j, h, :],
                    start=(k == 0),
                    stop=(k == n_mm - 1),
                )

            def emit_drain(ps, b, w, g):
                st = stage_pool.tile([128, HOP], f32, name=f"st{g}")
                if g % 2 == 0:
                    nc.vector.tensor_copy(st[:], ps[:])
                else:
                    nc.scalar.copy(st[:], ps[:])
                nc.sync.dma_start(out=out_d[b, 128 * w:128 * (w + 1), :],
                                  in_=st[:])

            for wave_start in range(0, len(groups), WAVE):
                wave = groups[wave_start:wave_start + WAVE]
                pss = [psum_pool.tile([128, HOP], f32, name=f"ps{g}",
                                      tag=f"ps{g}")
                       for g in range(len(wave))]
                if wave_start < 2 * WAVE:
                    k = 0
                    for j in range(TAPS):
                        for h in range(2):
                            for g, (b, w) in enumerate(wave):
                                emit_mm(pss[g], b, w, j, h, k)
                            k += 1
                    for g, (b, w) in enumerate(wave):
                        emit_drain(pss[g], b, w, g)
                else:
                    for g, (b, w) in enumerate(wave):
                        k = 0
                        for j in range(TAPS):
                            for h in range(2):
                                emit_mm(pss[g], b, w, j, h, k)
                                k += 1
                        emit_drain(pss[g], b, w, g)

    nc.compile()
    return nc


def _build_taps(proj, state_matrix, input_matrix, output_matrix,
                direct_matrix):
    P = np.asarray(proj, np.float64)
    A = np.asarray(state_matrix, np.float64)
    B = np.asarray(input_matrix, np.float64)
    C = np.asarray(output_matrix, np.float64)
    D = np.asarray(direct_matrix, np.float64)

    n = 2 * HOP
    w = 0.5 * (1.0 - np.cos(2.0 * np.pi * np.arange(n) / n))
    w1, w2 = w[:HOP], w[HOP:]

    V = [P @ D]
    PBAk = P @ B
    for _ in range(1, TAPS):
        PBAk = PBAk @ A
        V.append(PBAk @ C)

    U = np.zeros((TAPS, CPD, HOP))
    for j in range(TAPS):
        U[j] += V[j][:, :HOP] * w1
        if j >= 1:
            U[j] += V[j - 1][:, HOP:] * w2
    # k-tile layout: [cpd_within_half(128 part), tap, half, sample]
    return np.ascontiguousarray(
        U.reshape(TAPS, 2, 128, HOP).transpose(2, 0, 1, 3)
    ).astype(np.float32)


def kernel(control, proj, state_matrix, input_matrix, output_matrix,
           direct_matrix):
    from concourse.bass_utils import run_bass_kernel_spmd

    taps = _build_taps(proj, state_matrix, input_matrix, output_matrix,
                       direct_matrix)

    # control (B, CPD, F) -> per core [half, cpd128, b, frame+pad]
    c = np.asarray(control, np.float32)
    c = c.reshape(NCORES, BPC, 2, 128, FRAMES).transpose(0, 2, 3, 1, 4)
    ctrl_prep = np.zeros((NCORES, 2, 128, BPC, FP), np.float32)
    ctrl_prep[..., PAD:] = c

    if "nc" not in _CACHE:
        _CACHE["nc"] = _build_nc()
    nc = _CACHE["nc"]

    in_maps = [{"ctrl": ctrl_prep[i], "taps": taps} for i in range(NCORES)]
    res = run_bass_kernel_spmd(nc, in_maps, list(range(NCORES)))
    outs = [res.results[i]["out"] for i in range(NCORES)]
    sig = np.concatenate(outs, axis=0).reshape(BATCH, 1, FRAMES * HOP)
    return sig.astype(np.float32)


# revision 15
# speedup vs baseline: 1.2067x; 1.1808x over previous
"""SSM kernel for Trainium2 (8 NeuronCores, batch-data-parallel).

The reference network is a linear SSM:
    x_t  = ctrl_t^T P            (per frame t)
    xb_t = x_t B ;  xd_t = x_t D
    s_{t+1} = s_t A + xb_t ;  c_t = (s_t A) C
    out_t = c_t + xd_t  ->  hann window  ->  overlap-add (hop 512)

Because ||A||_2 ~ 0.37, contributions of xb_{t-k} to out_t decay ~5x per
step, so the whole network collapses (to ~1e-6 relative) into a short
causal convolution over frames whose taps absorb the projection, the
scan, the readout, the direct path, the hann window AND the overlap-add:

    signal[b, 512*t : 512*(t+1)] = sum_j ctrl[b, :, t-j]^T @ U_j

with U_j (256 x 512) precomputed on the host (weights-only math) from
V_0 = P@D, V_k = P@B@A^k@C:
    U_j = V_j[:, :512]*w1 + V_{j-1}[:, 512:]*w2.

On-device work is then a single dense matmul sweep: for each 128-frame
block the PSUM tile (128 frames x 512 samples) accumulates
TAPS x 2 matmuls with K=128 (cpd halves), N=512, using float32r
(full-rate FP22) and sliding lhsT windows into the SBUF-resident
control tensor. Batch is sharded 16-per-core across the 8 cores.
"""

import numpy as np

NCORES = 8
BATCH, CPD, FRAMES, HOP = 128, 256, 512, 512
BPC = BATCH // NCORES          # batch per core
TAPS = 5                       # conv depth (truncation err ~2.3e-4)
PAD = 16                       # zero frames prepended (>= TAPS-1)
FP = FRAMES + PAD
WBLK = FRAMES // 128           # 128-frame output blocks per batch elem

_CACHE = {}


def _build_nc():
    from concourse import bacc, tile, mybir

    f32 = mybir.dt.float32
    f32r = mybir.dt.float32r

    nc = bacc.Bacc("TRN2", target_bir_lowering=False, debug=False,
                   num_devices=NCORES)
    ctrl_d = nc.dram_tensor("ctrl", [2, 128, BPC, FP], f32r,
                            kind="ExternalInput").ap()
    taps_d = nc.dram_tensor("taps", [128, TAPS, 2, HOP], f32r,
                            kind="ExternalInput").ap()
    out_d = nc.dram_tensor("out", [BPC, FRAMES, HOP], f32,
                           kind="ExternalOutput").ap()

    with tile.TileContext(nc) as tc:
        with (
            tc.tile_pool(name="const", bufs=1) as const_pool,
            tc.tile_pool(name="psum", bufs=1, space="PSUM") as psum_pool,
            tc.tile_pool(name="stage", bufs=6) as stage_pool,
        ):
            # DMA issue order: tap j=0 first, then the two control batches
            # the first wave needs, then remaining taps in consumption
            # order, then the bulk of control.
            u_sb = const_pool.tile([128, TAPS, 2, HOP], f32r)
            ctrl_sb = const_pool.tile([128, 2, BPC, FP], f32r)
            def dma_u(j):
                for h in range(2):
                    nc.sync.dma_start(out=u_sb[:, j, h, :],
                                      in_=taps_d[:, j, h, :])

            def dma_ctrl(b, split=False):
                cuts = (0, 288, FP) if split else (0, FP)
                for lo, hi in zip(cuts[:-1], cuts[1:]):
                    for h in range(2):
                        nc.sync.dma_start(out=ctrl_sb[:, h, b, lo:hi],
                                          in_=ctrl_d[h, :, b, lo:hi])

            dma_u(0)
            dma_ctrl(0, split=True)
            dma_u(1)
            dma_ctrl(1, split=True)
            for j in range(2, TAPS):
                dma_u(j)
            for b in range(2, BPC):
                dma_ctrl(b)

            # Compute in waves of 8 concurrent PSUM groups. While inputs
            # are still streaming in (first 2 waves), emit matmuls
            # j-major across the wave so each arriving tap tile feeds 16
            # back-to-back matmuls and the ramp stays PE-bound. Once all
            # data is resident, emit group-major so PSUM drains spread
            # out evenly (shorter kernel tail).
            groups = [(b, w) for b in range(BPC) for w in range(WBLK)]
            WAVE = 8
            n_mm = TAPS * 2

            def emit_mm(ps, b, w, j, h, k):
                base = 128 * w + PAD
                nc.tensor.matmul(
                    ps[:],
                    ctrl_sb[:, h, b, base - j:base - j + 128],
                    u_sb[:, j, h, :],
                    start=(k == 0),
                    stop=(k == n_mm - 1),
                )

            def emit_drain(ps, b, w, g):
                st = stage_pool.tile([128, HOP], f32, name=f"st{g}")
                if g % 2 == 0:
                    nc.vector.tensor_copy(st[:], ps[:])
                else:
                    nc.scalar.copy(st[:], ps[:])
                nc.sync.dma_start(out=out_d[b, 128 * w:128 * (w + 1), :],
                                  in_=st[:])

            for wave_start in range(0, len(groups), WAVE):
                wave = groups[wave_start:wave_start + WAVE]
                pss = [psum_pool.tile([128, HOP], f32, name=f"ps{g}",
                                      tag=f"ps{g}")
                       for g in range(len(wave))]
                if wave_start < 2 * WAVE:
                    k = 0
                    for j in range(TAPS):
                        for h in range(2):
                            for g, (b, w) in enumerate(wave):
                                emit_mm(pss[g], b, w, j, h, k)
                            k += 1
                    for g, (b, w) in enumerate(wave):
                        emit_drain(pss[g], b, w, g)
                else:
                    for g, (b, w) in enumerate(wave):
                        k = 0
                        for j in range(TAPS):
                            for h in range(2):
                                emit_mm(pss[g], b, w, j, h, k)
                                k += 1
                        emit_drain(pss[g], b, w, g)

    nc.compile()
    return nc


def _build_taps(proj, state_matrix, input_matrix, output_matrix,
                direct_matrix):
    P = np.asarray(proj, np.float64)
    A = np.asarray(state_matrix, np.float64)
    B = np.asarray(input_matrix, np.float64)
    C = np.asarray(output_matrix, np.float64)
    D = np.asarray(direct_matrix, np.float64)

    n = 2 * HOP
    w = 0.5 * (1.0 - np.cos(2.0 * np.pi * np.arange(n) / n))
    w1, w2 = w[:HOP], w[HOP:]

    V = [P @ D]
    PBAk = P @ B
    for _ in range(1, TAPS):
        PBAk = PBAk @ A
        V.append(PBAk @ C)

    U = np.zeros((TAPS, CPD, HOP))
    for j in range(TAPS):
        U[j] += V[j][:, :HOP] * w1
        if j >= 1:
            U[j] += V[j - 1][:, HOP:] * w2
    # k-tile layout: [cpd_within_half(128 part), tap, half, sample]
    return np.ascontiguousarray(
        U.reshape(TAPS, 2, 128, HOP).transpose(2, 0, 1, 3)
    ).astype(np.float32)


def kernel(control, proj, state_matrix, input_matrix, output_matrix,
           direct_matrix):
    from concourse.bass_utils import run_bass_kernel_spmd

    taps = _build_taps(proj, state_matrix, input_matrix, output_matrix,
                       direct_matrix)

    # control (B, CPD, F) -> per core [half, cpd128, b, frame+pad]
    c = np.asarray(control, np.float32)
    c = c.reshape(NCORES, BPC, 2, 128, FRAMES).transpose(0, 2, 3, 1, 4)
    ctrl_prep = np.zeros((NCORES, 2, 128, BPC, FP), np.float32)
    ctrl_prep[..., PAD:] = c

    if "nc" not in _CACHE:
        _CACHE["nc"] = _build_nc()
    nc = _CACHE["nc"]

    in_maps = [{"ctrl": ctrl_prep[i], "taps": taps} for i in range(NCORES)]
    res = run_bass_kernel_spmd(nc, in_maps, list(range(NCORES)))
    outs = [res.results[i]["out"] for i in range(NCORES)]
    sig = np.concatenate(outs, axis=0).reshape(BATCH, 1, FRAMES * HOP)
    return sig.astype(np.float32)
